# revision 20
# baseline (speedup 1.0000x reference)
"""AffineTransform2D (spatial transformer, bilinear sampling) on 8 trn2 cores.

Fully device-side implementation (v2). Data parallel: 4 images per core.

Per core:
  Phase A (table build): the image set is reorganized in DRAM into a gather
    table `qtab` of 256B "window units": unit (r, s) holds the 2-row x 9-px
    x 3-ch f32 pixel window rows (r, r+1), columns (8s .. 8s+8), padded from
    216B to a 256B stride (dma_gather requires 256B-multiple units).
    Unit ids r*64+s <= 32703 fit the gather ucode's int16 indices.
  Phase B (per 128-row output block): affine coords X,Y are computed with
    the reference's exact fp32 formulas, exact floors via int32 round-trip +
    is_gt fixup, clamped taps, and the per-pixel unit id
    idx = y0c*64 + (x0c>>3). The [128,512] int16 id tile is reshuffled into
    the gather ucode's wrapped index layout (index k on partition k%16, slot
    k//16, replicated over the 8 gpsimd cores) by a DRAM bounce: one
    contiguous store + one strided re-read. Index order k = a*8192 + j*16 + b
    (output row i = 16a+b) makes both bounce DMAs contiguous.
  Phase C (per 32-row chunk): one dma_gather fetches 16384 windows (4MB)
    from qtab; the gathered data lands in a scrambled-but-regular layout
    [p = 16*(j%8) + i%16, slot = (i//32 %2)*64 + j//8]. Weights are computed
    directly in that layout (coords are affine in any (partition,slot)
    decomposition, bit-identical values). The x-selection uses exact hat
    weights u_d = relu(1 - |xl - d|), xl = X - 8*(x0c>>3): u equals the
    reference's bilinear wx0/wx1 at d = o, o+1 and is exactly 0 elsewhere.
    out_ch = sum_{r,d} (u_d * wy_r) * win[r, d, ch] via one product + one
    reduction per channel. The final DMA descrambles to row-major output.

Measured (marginal async-batch timing, device-resident inputs): full kernel
~13.1ms/core. Ablations: table+idx ~1.0ms; compute+out without gathers
~4.7ms; gathers without out-DMAs ~9.7ms; a standalone 256-long gather chain
with cold unique indices sustains ~3.2us/gather (~3ns/desc), so the gather
floor is ~3.3ms. The pieces do not overlap: ~5-6ms is cross-stream
serialization between the Pool gather stream and DVE compute / win-buffer
rotation that pool-depth changes (win bufs 3, acc bufs 6) did not fix —
needs trace-level visibility (unavailable: no NTFF hook) or a restructure
that splits compute to win-slice granularity. Out-descramble DMAs add
~3.4ms (1M 12-byte descriptor rows); fixing that needs a PE-transpose
descramble. Hard constraints found: the swdge ucode's 16KB descriptor
carveout caps one dma_gather at 1024 indices (2048 crashes the device);
every HWDGE vector-dynamic-offset variant probed crashes or silently uses
only the first offset per partition.
"""

import os
import sys
import time

import numpy as np

if "/opt/trn_rl_repo" not in sys.path:
    sys.path.insert(0, "/opt/trn_rl_repo")

import jax
from jax.sharding import Mesh, PartitionSpec, NamedSharding
from jax.experimental.shard_map import shard_map

from concourse import bacc, bass, mybir, library_config
import concourse.tile as tile
from concourse.bass2jax import (
    _bass_exec_p,
    install_neuronx_cc_hook,
    partition_id_tensor,
)

H = 512
W = 512
C = 3
HWPX = H * W
N_CORES = 8
IPC = 4                 # images per core
P = 128
NBLK = 4                # 128-row blocks per image
NCHUNK = 4              # 32-row chunks per block
SS = 64                 # x units per image row (8-px grid)
NU = 511 * SS           # units per image (32704, fits int16 ids)
UE = 64                 # f32 elems per unit (256B)
SC = 512.0 / 511.0

F32 = mybir.dt.float32
I32 = mybir.dt.int32
I16 = mybir.dt.int16
OP = mybir.AluOpType
ACT = mybir.ActivationFunctionType
KERNEL_VERSION = 16
VARIANT = os.environ.get("KVARIANT", "full")
DO_TABLE = VARIANT in ("full", "nocompute", "tbidx", "nogather")
DO_IDX = VARIANT in ("full", "nocompute", "tbidx", "nogather")
DO_GATHER = VARIANT in ("full", "nocompute")
DO_COMPUTE = VARIANT in ("full", "nogather")
DO_OUT = VARIANT in ("full", "nocompute", "nogather")
if VARIANT == "noout":
    DO_TABLE = DO_IDX = DO_GATHER = DO_COMPUTE = True
    DO_OUT = False
if VARIANT == "nogather2":
    DO_TABLE = DO_IDX = DO_COMPUTE = DO_OUT = True
    DO_GATHER = False


def A(ap, off, dims):
    """Custom access pattern on ap's tensor: element offset + [step, num] dims."""
    return bass.AP(ap.tensor, ap.offset + off, dims)


def _build_program():
    nc = bacc.Bacc("TRN2", num_swdge_queues=4)

    nc.dram_tensor("vtag", [1, KERNEL_VERSION], F32, kind="ExternalInput")
    im_d = nc.dram_tensor("im", [IPC * HWPX, C], F32, kind="ExternalInput")
    th_d = nc.dram_tensor("thetas", [P, IPC * 6], F32, kind="ExternalInput")
    ioj_d = nc.dram_tensor("iota_j", [P, W], F32, kind="ExternalInput")
    iop_d = nc.dram_tensor("iota_p", [P, 1], F32, kind="ExternalInput")
    ioi0_d = nc.dram_tensor("iota_i0", [P, P], F32, kind="ExternalInput")
    iojs_d = nc.dram_tensor("iota_js", [P, P], F32, kind="ExternalInput")
    qtab_d = nc.dram_tensor("qtab", [IPC * NU, UE], F32, kind="Internal")
    idxb_d = nc.dram_tensor("idxb", [2 * P * W], I16, kind="Internal")
    out_d = nc.dram_tensor("out", [IPC * HWPX, C], F32, kind="ExternalOutput")

    with tile.TileContext(nc) as tc:
        _body(nc, tc, im_d, th_d, ioj_d, iop_d, ioi0_d, iojs_d, qtab_d, idxb_d, out_d)
    nc.compile()
    return nc


def _body(nc, tc, im_d, th_d, ioj_d, iop_d, ioi0_d, iojs_d, qtab_d, idxb_d, out_d):
    tt = nc.vector.tensor_tensor
    ts = nc.vector.tensor_scalar
    tsm = nc.vector.tensor_scalar_mul
    stt = nc.vector.scalar_tensor_tensor
    tcp = nc.vector.tensor_copy

    im_ap = im_d[:, :]
    qtab_ap = qtab_d[:, :]
    idxb_ap = idxb_d[:]
    out_ap = out_d[:, :]

    with (
        tc.tile_pool(name="const", bufs=1) as cpool,
        tc.tile_pool(name="scal", bufs=1) as spool,
        tc.tile_pool(name="rt", bufs=1) as rtpool,
        tc.tile_pool(name="st", bufs=1) as stpool,
        tc.tile_pool(name="bp", bufs=1) as bpool,
        tc.tile_pool(name="wp", bufs=2) as wpool,
        tc.tile_pool(name="up", bufs=1) as upool,
        tc.tile_pool(name="vp", bufs=1) as vpool,
        tc.tile_pool(name="pp", bufs=1) as ppool,
        tc.tile_pool(name="ap", bufs=2) as apool,
        tc.tile_pool(name="gp", bufs=2) as gpool,
        tc.tile_pool(name="ip", bufs=2) as ipool,
    ):
        nc.gpsimd.load_library(library_config.mlp)

        # cache-buster scratch
        scratch = cpool.tile([1, 1], F32)
        nc.vector.memset(scratch[:], float(KERNEL_VERSION))

        iota_j = cpool.tile([P, W], F32)
        nc.sync.dma_start(iota_j[:], ioj_d[:])
        iota_p = cpool.tile([P, 1], F32)
        nc.sync.dma_start(iota_p[:], iop_d[:])
        ioi0 = cpool.tile([P, P], F32)
        nc.sync.dma_start(ioi0[:], ioi0_d[:])
        iojs = cpool.tile([P, P], F32)
        nc.sync.dma_start(iojs[:], iojs_d[:])
        th_all = cpool.tile([P, IPC * 6], F32)
        nc.sync.dma_start(th_all[:], th_d[:])

        # per-partition constants for Act-engine bias/scale operands
        actc = cpool.tile([P, 10], F32)
        for d in range(9):
            nc.vector.memset(actc[:, d:d + 1], float(-d))
        nc.vector.memset(actc[:, 9:10], -1.0)

        # ---- per-image affine scalars (exactly the reference fp32 math) ----
        # X(i,j) = cx + bx*i + ax*j ; Y likewise.
        scs = []
        for img in range(IPC):
            th = th_all[:, img * 6: (img + 1) * 6]
            sc = spool.tile([P, 8], F32, tag=f"sc{img}")
            ax, bx, cx = sc[:, 0:1], sc[:, 1:2], sc[:, 2:3]
            ay, by, cy = sc[:, 3:4], sc[:, 4:5], sc[:, 5:6]
            t0 = spool.tile([P, 2], F32, tag=f"t0{img}")
            tsm(ax, th[:, 0:1], SC)
            tsm(bx, th[:, 1:2], SC)
            tsm(ay, th[:, 3:4], SC)
            tsm(by, th[:, 4:5], SC)
            tt(t0[:, 0:1], th[:, 2:3], th[:, 0:1], OP.subtract)
            tt(t0[:, 0:1], t0[:, 0:1], th[:, 1:2], OP.subtract)
            ts(cx, t0[:, 0:1], 1.0, 256.0, OP.add, OP.mult)
            tt(t0[:, 1:2], th[:, 5:6], th[:, 3:4], OP.subtract)
            tt(t0[:, 1:2], t0[:, 1:2], th[:, 4:5], OP.subtract)
            ts(cy, t0[:, 1:2], 1.0, 256.0, OP.add, OP.mult)
            scs.append((ax, bx, cx, ay, by, cy))

        # ---- Phase A: window-unit table build ----
        # unit (r, s): [0:27] = im[r, 8s:8s+9, :], [27:54] = im[r+1, ...].
        for img in range(IPC if DO_TABLE else 0):
            for rc in range(4):
                npart = 128 if rc < 3 else 127
                rt = rtpool.tile([P, 2 * W * C], F32, tag="rt")
                prt = rt[:].ap[0][0]
                src = A(im_ap, (img * HWPX + rc * 128 * W) * C,
                        [[W * C, npart], [1, 2 * W * C]])
                nc.sync.dma_start(rt[0:npart, :], src)

                st = stpool.tile([P, SS * UE], F32, tag="st")
                pst = st[:].ap[0][0]
                # half 0: row r windows
                nc.vector.tensor_copy(
                    A(st[:], 0, [[pst, npart], [UE, SS], [1, 27]]),
                    A(rt[:], 0, [[prt, npart], [24, SS], [1, 27]]),
                )
                # half 1: row r+1 windows (s <= 62 full, s = 63 trimmed to 24)
                nc.vector.tensor_copy(
                    A(st[:], 27, [[pst, npart], [UE, SS - 1], [1, 27]]),
                    A(rt[:], W * C, [[prt, npart], [24, SS - 1], [1, 27]]),
                )
                nc.vector.tensor_copy(
                    A(st[:], 27 + (SS - 1) * UE, [[pst, npart], [1, 24]]),
                    A(rt[:], W * C + 24 * (SS - 1), [[prt, npart], [1, 24]]),
                )
                dst = A(qtab_ap, (img * NU + rc * 128 * SS) * UE,
                        [[SS * UE, npart], [1, SS * UE]])
                nc.scalar.dma_start(dst, st[0:npart, :])

        # ---- Phases B & C ----
        for img in range(IPC):
            ax, bx, cx, ay, by, cy = scs[img]
            qtab_img = A(qtab_ap, img * NU * UE, [[UE, NU], [1, UE]])
            for blk in range(NBLK):
                bounce = (img * NBLK + blk) % 2
                if not DO_IDX:
                    break

                # -- Phase B: per-pixel unit ids in output layout [i, j] --
                rb = bpool.tile([P, 4], F32, tag="rb")
                rowi, xb, yb = rb[:, 0:1], rb[:, 1:2], rb[:, 2:3]
                ts(rowi, iota_p[:], 1.0, float(P * blk), OP.mult, OP.add)
                tt(xb, rowi, bx, OP.mult)
                tt(xb, xb, cx, OP.add)
                tt(yb, rowi, by, OP.mult)
                tt(yb, yb, cy, OP.add)

                X = bpool.tile([P, W], F32, tag="X")
                Y = bpool.tile([P, W], F32, tag="Y")
                tt(X[:], iota_j[:], ax.to_broadcast([P, W]), OP.mult)
                tt(X[:], X[:], xb.to_broadcast([P, W]), OP.add)
                tt(Y[:], iota_j[:], ay.to_broadcast([P, W]), OP.mult)
                tt(Y[:], Y[:], yb.to_broadcast([P, W]), OP.add)

                # exact floors + clamps (int32 round-trip + is_gt fixup)
                xi = bpool.tile([P, W], I32, tag="xi")
                g = bpool.tile([P, W], F32, tag="g")
                x0c = bpool.tile([P, W], F32, tag="x0c")
                y0c = bpool.tile([P, W], F32, tag="y0c")
                ts(x0c[:], X[:], -2.0, 513.0, OP.max, OP.min)
                tcp(xi[:], x0c[:])
                tcp(x0c[:], xi[:])
                tt(g[:], x0c[:], X[:], OP.is_gt)
                tt(x0c[:], x0c[:], g[:], OP.subtract)
                ts(x0c[:], x0c[:], 0.0, 510.0, OP.max, OP.min)
                ts(y0c[:], Y[:], -2.0, 513.0, OP.max, OP.min)
                tcp(xi[:], y0c[:])
                tcp(y0c[:], xi[:])
                tt(g[:], y0c[:], Y[:], OP.is_gt)
                tt(y0c[:], y0c[:], g[:], OP.subtract)
                ts(y0c[:], y0c[:], 0.0, 510.0, OP.max, OP.min)
                # s = x0c >> 3 (exact floor of x0c/8)
                sf = bpool.tile([P, W], F32, tag="sf")
                tsm(sf[:], x0c[:], 0.125)
                tcp(xi[:], sf[:])
                tcp(g[:], xi[:])
                tt(x0c[:], g[:], sf[:], OP.is_gt)   # x0c reused as fixup flag
                tt(sf[:], g[:], x0c[:], OP.subtract)
                # idx = y0c*64 + s
                idxf = bpool.tile([P, W], F32, tag="idxf")
                stt(idxf[:], y0c[:], 64.0, sf[:], OP.mult, OP.add)
                idx16 = bpool.tile([P, W], I16, tag="idx16")
                tcp(idx16[:], idxf[:])

                # -- bounce: [i, j] -> wrapped gather-index layout --
                nc.sync.dma_start(
                    A(idxb_ap, bounce * P * W, [[W, P], [1, W]]), idx16[:]
                )
                idxw = ipool.tile([P, 32 * P], I16, tag="idxw")
                for grp in range(8):
                    nc.sync.dma_start(
                        idxw[16 * grp: 16 * (grp + 1), :],
                        A(idxb_ap, bounce * P * W,
                          [[W, 16], [16 * W, 8], [1, W]]),
                    )

                # -- Phase C: 4 chunks of 32 output rows --
                for c in range(NCHUNK):
                    win = gpool.tile([P, P, UE], F32, tag="win")
                    if not DO_GATHER:
                        # stand-in fill so compute timing is measurable alone
                        nc.gpsimd.memset(win[:].rearrange("p a b -> p (a b)"), 0.5)
                    # swdge descriptor carveout caps one gather at 1024 idxs
                    for gsub in range(16 if DO_GATHER else 0):
                        nc.gpsimd.dma_gather(
                            win[:, gsub * 8: (gsub + 1) * 8, :], qtab_img,
                            idxw[:, c * 1024 + gsub * 64: c * 1024 + (gsub + 1) * 64],
                            1024, 1024, UE,
                            queue_num=gsub % 4,
                        )

                    if not DO_COMPUTE:
                        acc = apool.tile([P, P * C], F32, tag="acc")
                        apart = acc[:].ap[0][0]
                        nc.vector.memset(acc[:], 0.0)
                        if DO_OUT:
                            base = (img * HWPX + (128 * blk + 32 * c) * W) * C
                            for jl8 in range(8):
                                for a2 in range(2):
                                    dst = A(out_ap,
                                            base + jl8 * C + a2 * 16 * W * C,
                                            [[W * C, 16], [8 * C, SS], [1, C]])
                                    srcp = A(acc[:], a2 * SS * C,
                                             [[apart, 16], [1, SS * C]])
                                    srcp = bass.AP(srcp.tensor,
                                                   srcp.offset + jl8 * 16 * apart,
                                                   srcp.ap)
                                    q = nc.sync if (jl8 + a2) % 2 == 0 else nc.scalar
                                    q.dma_start(dst, srcp)
                        continue

                    # coords in the scrambled chunk layout (bit-identical values)
                    it = wpool.tile([P, P], F32, tag="it")
                    Xt = wpool.tile([P, P], F32, tag="Xt")
                    Yt = wpool.tile([P, P], F32, tag="Yt")
                    tmp = wpool.tile([P, P], F32, tag="tmp")
                    ts(it[:], ioi0[:], 1.0, float(128 * blk + 32 * c),
                       OP.mult, OP.add)
                    tt(tmp[:], it[:], bx.to_broadcast([P, P]), OP.mult)
                    tt(tmp[:], tmp[:], cx.to_broadcast([P, P]), OP.add)
                    tt(Xt[:], iojs[:], ax.to_broadcast([P, P]), OP.mult)
                    tt(Xt[:], Xt[:], tmp[:], OP.add)
                    tt(tmp[:], it[:], by.to_broadcast([P, P]), OP.mult)
                    tt(tmp[:], tmp[:], cy.to_broadcast([P, P]), OP.add)
                    tt(Yt[:], iojs[:], ay.to_broadcast([P, P]), OP.mult)
                    tt(Yt[:], Yt[:], tmp[:], OP.add)

                    xi2 = wpool.tile([P, P], I32, tag="xi2")
                    g2 = wpool.tile([P, P], F32, tag="g2")
                    x0f = wpool.tile([P, P], F32, tag="x0f")
                    y0f = wpool.tile([P, P], F32, tag="y0f")
                    x0ct = wpool.tile([P, P], F32, tag="x0ct")
                    y0ct = wpool.tile([P, P], F32, tag="y0ct")
                    m = wpool.tile([P, P], F32, tag="m")
                    ts(x0f[:], Xt[:], -2.0, 513.0, OP.max, OP.min)
                    tcp(xi2[:], x0f[:])
                    tcp(x0f[:], xi2[:])
                    tt(g2[:], x0f[:], Xt[:], OP.is_gt)
                    tt(x0f[:], x0f[:], g2[:], OP.subtract)
                    ts(y0f[:], Yt[:], -2.0, 513.0, OP.max, OP.min)
                    tcp(xi2[:], y0f[:])
                    tcp(y0f[:], xi2[:])
                    tt(g2[:], y0f[:], Yt[:], OP.is_gt)
                    tt(y0f[:], y0f[:], g2[:], OP.subtract)
                    ts(x0ct[:], x0f[:], 0.0, 510.0, OP.max, OP.min)
                    ts(y0ct[:], y0f[:], 0.0, 510.0, OP.max, OP.min)
                    tt(m[:], x0ct[:], x0f[:], OP.is_equal)
                    tt(g2[:], y0ct[:], y0f[:], OP.is_equal)
                    tt(m[:], m[:], g2[:], OP.mult)

                    wy0 = wpool.tile([P, P], F32, tag="wy0")
                    wy1 = wpool.tile([P, P], F32, tag="wy1")
                    stt(wy0[:], y0f[:], 1.0, Yt[:], OP.add, OP.subtract)
                    tt(wy1[:], Yt[:], y0f[:], OP.subtract)
                    tt(wy0[:], wy0[:], m[:], OP.mult)
                    tt(wy1[:], wy1[:], m[:], OP.mult)

                    # xl = X - 8*floor(x0c/8)
                    sf2 = wpool.tile([P, P], F32, tag="sf2")
                    xl = wpool.tile([P, P], F32, tag="xl")
                    tsm(sf2[:], x0ct[:], 0.125)
                    tcp(xi2[:], sf2[:])
                    tcp(g2[:], xi2[:])
                    tt(m[:], g2[:], sf2[:], OP.is_gt)  # m reused as fixup flag
                    tt(sf2[:], g2[:], m[:], OP.subtract)
                    stt(xl[:], sf2[:], -8.0, Xt[:], OP.mult, OP.add)

                    # hat weights on the Act engine: u_d = relu(1 - |xl - d|)
                    u = upool.tile([P, 9 * P], F32, tag="u")
                    for d in range(9):
                        td = upool.tile([P, P], F32, tag=f"td{d % 2}")
                        nc.scalar.activation(td[:], xl[:], ACT.Abs,
                                             bias=actc[:, d:d + 1], scale=1.0)
                        nc.scalar.activation(u[:, d * P: (d + 1) * P], td[:],
                                             ACT.Relu, bias=1.0,
                                             scale=actc[:, 9:10])

                    # v[r, d] = u_d * wy_r  (layout: elem = (r*9+d)*128 + s)
                    v = vpool.tile([P, 2 * 9 * P], F32, tag="v")
                    for r in range(2):
                        wyr = wy0 if r == 0 else wy1
                        for d in range(9):
                            tt(v[:, (r * 9 + d) * P: (r * 9 + d + 1) * P],
                               u[:, d * P: (d + 1) * P], wyr[:], OP.mult)

                    # per channel: prod = win * v (walk s,r,d), reduce over (r,d)
                    prod = ppool.tile([P, 18 * P], F32, tag="prod")
                    acc = apool.tile([P, P * C], F32, tag="acc")
                    ppart = prod[:].ap[0][0]
                    wpart = win[:].ap[0][0]
                    vpart = v[:].ap[0][0]
                    apart = acc[:].ap[0][0]
                    # split per half-chunk: half-0 products need only
                    # gathers 0..7, so DVE starts while 8..15 still drain
                    for h in range(2):
                        for ch in range(C):
                            tt(
                                A(prod[:], h * 64 * 18,
                                  [[ppart, P], [18, 64], [9, 2], [1, 9]]),
                                A(win[:], ch + h * 64 * UE,
                                  [[wpart, P], [UE, 64], [27, 2], [3, 9]]),
                                A(v[:], h * 64,
                                  [[vpart, P], [1, 64], [9 * P, 2], [P, 9]]),
                                OP.mult,
                            )
                            nc.vector.tensor_reduce(
                                A(acc[:], ch + h * 64 * C, [[apart, P], [C, 64]]),
                                A(prod[:], h * 64 * 18,
                                  [[ppart, P], [18, 64], [1, 18]]),
                                axis=mybir.AxisListType.X,
                                op=OP.add,
                            )

                    # descrambling store to row-major output
                    # (3-dim DMA AP limit: split per (jl8, a2))
                    base = (img * HWPX + (128 * blk + 32 * c) * W) * C
                    for jl8 in range(8 if DO_OUT else 0):
                        for a2 in range(2):
                            dst = A(out_ap, base + jl8 * C + a2 * 16 * W * C,
                                    [[W * C, 16], [8 * C, SS], [1, C]])
                            srcp = A(acc[:], a2 * SS * C,
                                     [[apart, 16], [1, SS * C]])
                            srcp = bass.AP(srcp.tensor,
                                           srcp.offset + jl8 * 16 * apart,
                                           srcp.ap)
                            q = nc.sync if (jl8 + a2) % 2 == 0 else nc.scalar
                            q.dma_start(dst, srcp)


# ---------------- host side: persistent jit + staging ----------------

_NC = None
_FN = None
_MESH = None
_META = None          # (in_names, out_names, zero_shapes)
_CONST_DEV = None     # cached device buffers for inputs that never change
_LAST_DEV_ARGS = None


def _get_nc():
    global _NC
    if _NC is None:
        _NC = _build_program()
    return _NC


def _make_fn(nc, n_cores):
    install_neuronx_cc_hook()
    in_names, out_names, out_avals, zero_outs = [], [], [], []
    for alloc in nc.m.functions[0].allocations:
        if not isinstance(alloc, mybir.MemoryLocationSet):
            continue
        name = alloc.memorylocations[0].name
        if alloc.kind == "ExternalInput":
            if nc.partition_id_tensor is None or name != nc.partition_id_tensor.name:
                in_names.append(name)
        elif alloc.kind == "ExternalOutput":
            out_names.append(name)
            shape = tuple(alloc.tensor_shape)
            dtype = mybir.dt.np(alloc.dtype)
            out_avals.append(jax.core.ShapedArray(shape, dtype))
            zero_outs.append(np.zeros(shape, dtype))
    n_params = len(in_names)
    all_names = list(in_names) + list(out_names)
    if nc.partition_id_tensor is not None:
        all_names.append(nc.partition_id_tensor.name)

    def _bodyf(*args):
        operands = list(args)
        if nc.partition_id_tensor is not None:
            operands.append(partition_id_tensor())
        outs = _bass_exec_p.bind(
            *operands,
            out_avals=tuple(out_avals),
            in_names=tuple(all_names),
            out_names=tuple(out_names),
            lowering_input_output_aliases=(),
            sim_require_finite=True,
            sim_require_nnan=True,
            nc=nc,
        )
        return tuple(outs)

    devices = jax.devices()[:n_cores]
    mesh = Mesh(np.asarray(devices), ("core",))
    n_outs = len(out_names)
    in_specs = (PartitionSpec("core"),) * (n_params + n_outs)
    out_specs = (PartitionSpec("core"),) * n_outs
    fn = jax.jit(
        shard_map(_bodyf, mesh=mesh, in_specs=in_specs, out_specs=out_specs,
                  check_rep=False),
        keep_unused=True,
    )
    return fn, mesh, in_names, out_names, zero_outs


def _get_fn():
    global _FN, _MESH, _META
    if _FN is None:
        nc = _get_nc()
        _FN, _MESH, in_names, out_names, zero_outs = _make_fn(nc, N_CORES)
        _META = (in_names, out_names, zero_outs)
    return _FN, _MESH, _META


def _host_constants():
    iota_j = np.broadcast_to(
        np.arange(W, dtype=np.float32)[None, :], (P, W)).copy()
    iota_p = np.arange(P, dtype=np.float32)[:, None].copy()
    pp, ss = np.meshgrid(np.arange(P), np.arange(P), indexing="ij")
    ioi0 = (16 * (ss // 64) + pp % 16).astype(np.float32)
    iojs = (8 * (ss % 64) + pp // 16).astype(np.float32)
    return iota_j, iota_p, ioi0, iojs


def stage_inputs(im, thetas):
    """Concat per-core inputs into the global sharded arrays (numpy)."""
    im = np.ascontiguousarray(np.asarray(im, dtype=np.float32))
    thetas = np.ascontiguousarray(np.asarray(thetas, dtype=np.float32))
    iota_j, iota_p, ioi0, iojs = _host_constants()
    fn, mesh, (in_names, out_names, zero_outs) = _get_fn()

    per_core = {
        "vtag": [], "im": [], "thetas": [], "iota_j": [], "iota_p": [],
        "iota_i0": [], "iota_js": [],
    }
    for core in range(N_CORES):
        sl = slice(core * IPC, (core + 1) * IPC)
        th_rep = np.broadcast_to(
            thetas[sl].reshape(1, IPC * 6), (P, IPC * 6)).copy()
        per_core["vtag"].append(np.zeros((1, KERNEL_VERSION), np.float32))
        per_core["im"].append(im[sl].reshape(IPC * HWPX, C))
        per_core["thetas"].append(th_rep)
        per_core["iota_j"].append(iota_j)
        per_core["iota_p"].append(iota_p)
        per_core["iota_i0"].append(ioi0)
        per_core["iota_js"].append(iojs)

    args = [np.concatenate(per_core[name], axis=0) for name in in_names]
    zeros = [np.zeros((N_CORES * z.shape[0], *z.shape[1:]), z.dtype)
             for z in zero_outs]
    return args, zeros


def stage_device(im, thetas):
    """device_put the staged inputs, caching the constant ones."""
    global _CONST_DEV
    fn, mesh, (in_names, out_names, zero_outs) = _get_fn()
    sh = NamedSharding(mesh, PartitionSpec("core"))
    args, zeros = stage_inputs(im, thetas)
    if _CONST_DEV is None:
        _CONST_DEV = {}
        for name, arr in zip(in_names, args):
            if name not in ("im", "thetas"):
                _CONST_DEV[name] = jax.device_put(arr, sh)
        _CONST_DEV["__zeros__"] = [jax.device_put(z, sh) for z in zeros]
    dev_args = []
    for name, arr in zip(in_names, args):
        if name in ("im", "thetas"):
            dev_args.append(jax.device_put(arr, sh))
        else:
            dev_args.append(_CONST_DEV[name])
    dev_args.extend(_CONST_DEV["__zeros__"])
    jax.block_until_ready(dev_args)
    return dev_args


def kernel(im, mb_size, thetas):
    global _LAST_DEV_ARGS
    t0 = time.time()
    fn, mesh, (in_names, out_names, zero_outs) = _get_fn()
    dev_args = stage_device(im, thetas)
    _LAST_DEV_ARGS = dev_args
    t1 = time.time()
    outs = fn(*dev_args)
    jax.block_until_ready(outs)
    t2 = time.time()
    out_g = np.asarray(outs[0])
    t3 = time.time()
    print(f"[kernel] stage+H2D {t1-t0:.2f}s exec {t2-t1:.2f}s D2H {t3-t2:.2f}s")
    return out_g.reshape(N_CORES * IPC, H, W, C)
